# revision 9
# baseline (speedup 1.0000x reference)
"""Trainium2 Bass kernel for nn_FFM_71347996721190.

Model: gated complex linear recurrence (FFM) + dense in/out projections +
gated LayerNorm residual.  T=8192 timesteps are sharded across 8 NeuronCores
(sequence parallel, 1024 steps per core); each core runs the full pipeline on
its chunk with zero carried-in state, and the tiny cross-chunk carry fix-up
(prefix rows before each chunk's first episode reset) is applied on the host.

Device algorithm per core (chunk length L=1024):
  gate_in/pre      : f32r matmuls  x @ {gi,pre}_w.T (packed in one PSUM)
  gated_x          : sigmoid + multiply   -> gxT [64, L]
  scan             : rotating-frame trick.  s_t = gamma*s_{t-1}+x_t with
                     gamma = exp(-|a| + i*b) decouples into two REAL scans
                     u_t = e^{-a} u_{t-1} + cos(b t) x_t  (and -sin for imag)
                     via the DVE tensor_tensor_scan instruction, with episode
                     resets folded into the per-step multiplier A_t
                     (A_t = keep_t * e^{-a}).  Recover s = e^{i b t} u with
                     elementwise cos/sin tables (host-precomputed in f64).
                     Channel layout: partition p of group g <-> trace
                     m = 8g + p//16, context c = p%16  (8 groups x 128).
  z                : f32r matmul  [Re(s);Im(s)] (channel-major, stationary)
                     @ mix_w.T (moving), accumulated over 16 k-chunks
  gate_out/skip    : f32r matmuls with x^T tiles stationary
  epilogue         : zg = z*sigmoid(go); LayerNorm(zg); + skip*(1-sigmoid(go))

Everything is emitted through TileContext (auto-sync).  Two environment
workarounds are inlined: the walrus build here rejects >1 semaphore wait per
instruction (waits are hoisted onto single-wait NoOps in a BIR post-pass) and
the TileContext tail drain carries many waits (same fix at emission time).
"""

import contextlib
import json
import numpy as np

import bass_rust
import concourse.bass as bass
import concourse.mybir as mybir
from concourse import tile
from concourse import bass2jax

# ----------------------------------------------------------------------------
# problem constants (hardcoded per harness contract)
# ----------------------------------------------------------------------------
T = 8192
D = 1024          # INPUT == OUT
M = 64            # trace
C = 16            # context
NCORES = 8
L = T // NCORES   # 1024 timesteps per core
P = 128
EPS = 1e-5
F32 = mybir.dt.float32
F32R = mybir.dt.float32r
AL = mybir.AluOpType
AF = mybir.ActivationFunctionType

NG = 8            # scan partition groups (8 x 128 = 1024 (m,c) channels)
NTB = L // P      # 8 time blocks per chunk
KD = D // P       # 8 k-chunks over INPUT
KCH = 16          # k-chunks over 2*M*C for mix

# ----------------------------------------------------------------------------
# environment workarounds
# ----------------------------------------------------------------------------
_patched = False


def _patched_drain_and_barrier(self, tick_clock, wait_clock):
    # stock version puts every outstanding sem wait on one Drain; this
    # walrus build allows at most one sync wait per instruction.
    nc = self.nc
    vals = list(tick_clock.global_clock)
    nc.sync.drain()
    for i, v in enumerate(vals):
        if v:
            single = [0] * len(vals)
            single[i] = v
            n = nc.sync.nop(nofuse=True)
            wait_clock.add_sem_waits(
                n.ins, tile.ScopedClock({None: bass_rust.VectorClock(single)})
            )
    nc.all_engine_barrier()
    assert self.sems is not None
    popped = nc._tile_sem_poison_stack.pop()
    assert popped is self._sem_poison
    nc.clear_and_free_semaphores(list(self.sems.allocated().values()))
    nc.all_engine_barrier()


_wsplit_ctr = [0]


def _fix_bir_json(d):
    """Hoist multi-waits (and all waits on PE compute instructions, whose
    ldweights lowering slot takes none) onto dedicated single-wait NoOps."""
    for fn in d.get("functions", []):
        for blk in fn.get("blocks", []):
            out = []
            for inst in blk.get("instructions", []):
                si = inst.get("sync_info")
                waits = (si or {}).get("on_wait") or []
                engine = inst.get("engine")
                is_pe_compute = engine == "PE" and inst.get("opcode") not in (
                    "NoOp", "Drain", "EventSemaphore")
                if len(waits) > 1 or (is_pe_compute and len(waits) >= 1):
                    for w in waits:
                        _wsplit_ctr[0] += 1
                        nop = {
                            "engine": engine,
                            "ins": [],
                            "outs": [],
                            "name": f"I-wsplit-{_wsplit_ctr[0]}",
                            "opcode": "NoOp",
                            "sync_info": {"on_wait": [w], "on_update": []},
                        }
                        if "debug" in inst:
                            nop["debug"] = inst["debug"]
                        out.append(nop)
                    si["on_wait"] = []
                out.append(inst)
            blk["instructions"] = out


def _install_patches():
    global _patched
    if _patched:
        return
    tile.TileContext._drain_and_barrier = _patched_drain_and_barrier

    orig = bass2jax.compile_bir_kernel

    def wrapped(ant_bir_str, compile_dir_path, neff_name=None, **kw):
        d = json.loads(ant_bir_str)
        _fix_bir_json(d)
        return orig(json.dumps(d).encode(), compile_dir_path,
                    neff_name=neff_name, **kw)

    bass2jax.compile_bir_kernel = wrapped
    _patched = True


# ----------------------------------------------------------------------------
# bass program (identical for all 8 cores)
# ----------------------------------------------------------------------------

def build_program():
    nc = bass.Bass()

    xt_d = nc.dram_tensor("xt", [D, L], F32R, kind="ExternalInput")
    keep_d = nc.dram_tensor("keep", [1, L], F32R, kind="ExternalInput")
    costab_d = nc.dram_tensor("costab", [P, L], F32, kind="ExternalInput")
    nsintab_d = nc.dram_tensor("nsintab", [P, L], F32, kind="ExternalInput")
    dec_d = nc.dram_tensor("dec", [P, 8], F32, kind="ExternalInput")
    emat_d = nc.dram_tensor("emat", [M, 8 * P], F32R, kind="ExternalInput")
    ones1_d = nc.dram_tensor("ones1", [1, P], F32R, kind="ExternalInput")
    pgw_d = nc.dram_tensor("pgw", [D, P], F32R, kind="ExternalInput")
    pgb_d = nc.dram_tensor("pgb", [P, 1], F32, kind="ExternalInput")
    gow_d = nc.dram_tensor("gow", [D, D], F32R, kind="ExternalInput")
    skw_d = nc.dram_tensor("skw", [D, D], F32R, kind="ExternalInput")
    mixw_d = nc.dram_tensor("mixw", [2 * M * C, D], F32R, kind="ExternalInput")
    gob_d = nc.dram_tensor("gob", [1, D], F32R, kind="ExternalInput")
    skb_d = nc.dram_tensor("skb", [1, D], F32R, kind="ExternalInput")
    mixb_d = nc.dram_tensor("mixb", [1, D], F32R, kind="ExternalInput")

    out_d = nc.dram_tensor("out", [L, D], F32, kind="ExternalOutput")
    gx_d = nc.dram_tensor("gx", [M, L], F32R, kind="ExternalOutput")

    with tile.TileContext(nc) as tc:
        with (
            tc.tile_pool(name="const", bufs=1) as cpool,
            tc.tile_pool(name="s", bufs=1) as spool,
        ):
            xt_stack = contextlib.ExitStack()
            xpool = xt_stack.enter_context(
                tc.tile_pool(name="xtp", bufs=1, side="right"))
            # --- long-lived tiles ---
            ones1 = cpool.tile([1, P], F32R, tag="ones1")
            gob_t = cpool.tile([1, D], F32R, tag="gob")
            skb_t = cpool.tile([1, D], F32R, tag="skb")
            mixb_t = cpool.tile([1, D], F32R, tag="mixb")
            gx = cpool.tile([M, L], F32R, tag="gxs")
            nc.sync.dma_start(ones1[:], ones1_d[:])
            nc.sync.dma_start(gob_t[:], gob_d[:])
            nc.sync.dma_start(skb_t[:], skb_d[:])
            nc.sync.dma_start(mixb_t[:], mixb_d[:])

            s_re = [spool.tile([P, L], F32R, tag=f"sre{g}", name=f"sre{g}")
                    for g in range(NG)]
            s_im = [spool.tile([P, L], F32R, tag=f"sim{g}", name=f"sim{g}")
                    for g in range(NG)]

            xt = xpool.tile([P, KD * L], F32R, tag="xt")
            for kc in range(KD):
                nc.sync.dma_start(xt[:, kc * L:(kc + 1) * L],
                                  xt_d[kc * P:(kc + 1) * P, :])

            # =============== phase 1-3: scan path ===============
            with (
                tc.tile_pool(name="scanc", bufs=1, side="right") as scpool,
                tc.tile_pool(name="psA", bufs=1, space="PSUM") as psA,
                tc.tile_pool(name="psR", bufs=2, space="PSUM") as psR,
                tc.tile_pool(name="scant", bufs=2, side="right") as tpool,
                tc.tile_pool(name="u", bufs=2, side="right") as upool,
                tc.tile_pool(name="p1", bufs=1, side="right") as p1pool,
            ):
                costab = scpool.tile([P, L], F32, tag="costab")
                nsintab = scpool.tile([P, L], F32, tag="nsintab")
                dec = scpool.tile([P, 8], F32, tag="dec")
                emat = scpool.tile([M, 8 * P], F32R, tag="emat")
                pgw = scpool.tile([P, D], F32R, tag="pgw")
                pgb = scpool.tile([P, 1], F32, tag="pgb")
                keep = scpool.tile([1, L], F32R, tag="keep")
                keeprep = scpool.tile([P, L], F32, tag="keeprep")
                nc.sync.dma_start(costab[:], costab_d[:])
                nc.sync.dma_start(nsintab[:], nsintab_d[:])
                nc.sync.dma_start(dec[:], dec_d[:])
                nc.sync.dma_start(emat[:], emat_d[:])
                nc.sync.dma_start(pgb[:], pgb_d[:])
                nc.sync.dma_start(keep[:], keep_d[:])
                for kc in range(KD):
                    nc.sync.dma_start(pgw[:, kc * P:(kc + 1) * P],
                                      pgw_d[kc * P:(kc + 1) * P, :])

                pgps = psA.tile([P, L], F32, tag="pg")
                for th in range(2):
                    sl = slice(th * 512, (th + 1) * 512)
                    for kc in range(KD):
                        nc.tensor.matmul(
                            pgps[:, sl],
                            lhsT=pgw[:, kc * P:(kc + 1) * P],
                            rhs=xt[:, kc * L + th * 512: kc * L + (th + 1) * 512],
                            start=(kc == 0), stop=(kc == KD - 1))
                pg = p1pool.tile([P, L], F32, tag="pg1")
                gi2 = p1pool.tile([M, L], F32, tag="gi2")
                # pre rows 0:64 (+bias), sigmoid(gi) rows 64:128 (+bias)
                nc.scalar.activation(pg[0:M, :], pgps[0:M, :], AF.Identity,
                                     bias=pgb[0:M, 0:1])
                nc.scalar.activation(pg[M:P, :], pgps[M:P, :], AF.Sigmoid,
                                     bias=pgb[M:P, 0:1])
                nc.sync.dma_start(gi2[:], pg[M:P, :])
                nc.vector.tensor_tensor(gx[:], pg[0:M, :], gi2[:], AL.mult)
                nc.sync.dma_start(gx_d[:], gx[:])

                # keep replication to all 128 partitions
                kps = psA.tile([P, L], F32, tag="keep")
                for th in range(2):
                    nc.tensor.matmul(kps[:, th * 512:(th + 1) * 512],
                                     lhsT=ones1[:],
                                     rhs=keep[:, th * 512:(th + 1) * 512],
                                     start=True, stop=True)
                nc.scalar.copy(keeprep[:], kps[:])

                # per group: replicate gx, scan, recover s
                for g in range(NG):
                    rps = psR.tile([P, L], F32, tag="repl")
                    for th in range(2):
                        nc.tensor.matmul(
                            rps[:, th * 512:(th + 1) * 512],
                            lhsT=emat[:, g * P:(g + 1) * P],
                            rhs=gx[:, th * 512:(th + 1) * 512],
                            start=True, stop=True)
                    b_re = tpool.tile([P, L], F32, tag="bre")
                    b_im = tpool.tile([P, L], F32, tag="bim")
                    a_g = tpool.tile([P, L], F32, tag="ag")
                    nc.vector.tensor_tensor(b_re[:], rps[:], costab[:], AL.mult)
                    nc.vector.tensor_tensor(b_im[:], rps[:], nsintab[:], AL.mult)
                    nc.vector.tensor_scalar(a_g[:], keeprep[:], dec[:, g:g + 1],
                                            None, op0=AL.mult)
                    u_re = upool.tile([P, L], F32, tag="ure")
                    u_im = upool.tile([P, L], F32, tag="uim")
                    nc.vector.tensor_tensor_scan(u_re[:], a_g[:], b_re[:], 0.0,
                                                 op0=AL.mult, op1=AL.add)
                    nc.vector.tensor_tensor_scan(u_im[:], a_g[:], b_im[:], 0.0,
                                                 op0=AL.mult, op1=AL.add)
                    # s_re = cos*u_re + (-sin)*u_im
                    # s_im = cos*u_im - (-sin)*u_re
                    t1 = tpool.tile([P, L], F32, tag="t1")
                    t2 = tpool.tile([P, L], F32, tag="t2")
                    nc.vector.tensor_tensor(t1[:], costab[:], u_re[:], AL.mult)
                    nc.vector.tensor_tensor(t2[:], nsintab[:], u_im[:], AL.mult)
                    nc.vector.tensor_tensor(s_re[g][:], t1[:], t2[:], AL.add)
                    nc.vector.tensor_tensor(t1[:], costab[:], u_im[:], AL.mult)
                    nc.vector.tensor_tensor(t2[:], nsintab[:], u_re[:], AL.mult)
                    nc.vector.tensor_tensor(s_im[g][:], t1[:], t2[:], AL.subtract)

            # =============== phase 4: gate_out / skip ===============
            with tc.tile_pool(name="gos", bufs=1) as gpool:
              go_sig = [gpool.tile([P, D], F32, tag=f"gosig{tb}",
                                   name=f"gosig{tb}") for tb in range(NTB)]
              sk1 = [gpool.tile([P, D], F32, tag=f"sk1{tb}",
                                name=f"sk1{tb}") for tb in range(NTB)]
              with (
                tc.tile_pool(name="psB4", bufs=1, space="PSUM") as psB4,
                tc.tile_pool(name="w4", bufs=2, side="right") as wpool4,
                tc.tile_pool(name="gneg", bufs=2, side="right") as npool,
              ):
                for tb in range(NTB):
                    bank = [psB4.tile([P, 512], F32, tag=f"bank{i}", bufs=2,
                                      name=f"bank4_{tb}_{i}") for i in range(4)]
                    for h in range(2):
                        nc.tensor.matmul(
                            bank[h][:], lhsT=ones1[:],
                            rhs=gob_t[:, h * 512:(h + 1) * 512],
                            start=True, stop=False)
                        nc.tensor.matmul(
                            bank[2 + h][:], lhsT=ones1[:],
                            rhs=skb_t[:, h * 512:(h + 1) * 512],
                            start=True, stop=False)
                    for kc in range(KD):
                        lh = xt[:, kc * L + tb * P: kc * L + (tb + 1) * P]
                        gw = wpool4.tile([P, 512], F32R, tag="w4g0")
                        gw1 = wpool4.tile([P, 512], F32R, tag="w4g1")
                        sw = wpool4.tile([P, 512], F32R, tag="w4s0")
                        sw1 = wpool4.tile([P, 512], F32R, tag="w4s1")
                        nc.sync.dma_start(gw[:], gow_d[kc * P:(kc + 1) * P, 0:512])
                        nc.sync.dma_start(gw1[:], gow_d[kc * P:(kc + 1) * P, 512:D])
                        nc.sync.dma_start(sw[:], skw_d[kc * P:(kc + 1) * P, 0:512])
                        nc.sync.dma_start(sw1[:], skw_d[kc * P:(kc + 1) * P, 512:D])
                        last = kc == KD - 1
                        nc.tensor.matmul(bank[0][:], lhsT=lh, rhs=gw[:],
                                         start=False, stop=last)
                        nc.tensor.matmul(bank[1][:], lhsT=lh, rhs=gw1[:],
                                         start=False, stop=last)
                        nc.tensor.matmul(bank[2][:], lhsT=lh, rhs=sw[:],
                                         start=False, stop=last)
                        nc.tensor.matmul(bank[3][:], lhsT=lh, rhs=sw1[:],
                                         start=False, stop=last)
                    for h in range(2):
                        hs = slice(h * 512, (h + 1) * 512)
                        nc.scalar.activation(go_sig[tb][:, hs], bank[h][:],
                                             AF.Sigmoid)
                        gneg = npool.tile([P, 512], F32, tag="gneg")
                        nc.vector.tensor_scalar(gneg[:], go_sig[tb][:, hs],
                                                -1.0, 1.0, op0=AL.mult,
                                                op1=AL.add)
                        nc.vector.scalar_tensor_tensor(
                            sk1[tb][:, hs], bank[2 + h][:], 0.0, gneg[:],
                            op0=AL.bypass, op1=AL.mult)

              # xt no longer needed; free its SBUF before the mix phase
              xt_stack.close()
              # =============== phase 5: mix matmul + zg ===============
              if True:
                with (
                    tc.tile_pool(name="psB5", bufs=1, space="PSUM") as psB5,
                    tc.tile_pool(name="w5", bufs=4) as wpool5,
                    tc.tile_pool(name="zgp", bufs=1) as zpool,
                    tc.tile_pool(name="ep", bufs=2) as epool,
                ):
                    zg = [zpool.tile([P, D], F32, tag=f"zg{tb}", name=f"zg{tb}")
                          for tb in range(NTB)]
                    s1 = [zpool.tile([P, 2], F32, tag=f"s1_{tb}", name=f"s1_{tb}")
                          for tb in range(NTB)]
                    for h in range(2):
                        hs = slice(h * 512, (h + 1) * 512)
                        zb = [psB5.tile([P, 512], F32, tag=f"bank{i}",
                                        name=f"bank5_{h}_{i}") for i in range(NTB)]
                        for tb in range(NTB):
                            nc.tensor.matmul(
                                zb[tb][:], lhsT=ones1[:],
                                rhs=mixb_t[:, h * 512:(h + 1) * 512],
                                start=True, stop=False)
                        for cc in range(KCH):
                            mw = wpool5.tile([P, 512], F32R, tag="w5")
                            nc.sync.dma_start(mw[:], mixw_d[cc * P:(cc + 1) * P, hs])
                            stile = s_re[cc] if cc < NG else s_im[cc - NG]
                            for tb in range(NTB):
                                nc.tensor.matmul(
                                    zb[tb][:],
                                    lhsT=stile[:, tb * P:(tb + 1) * P],
                                    rhs=mw[:],
                                    start=False, stop=(cc == KCH - 1))
                        for tb in range(NTB):
                            nc.vector.scalar_tensor_tensor(
                                zg[tb][:, hs], zb[tb][:], 0.0, go_sig[tb][:, hs],
                                op0=AL.bypass, op1=AL.mult,
                                accum_out=s1[tb][:, h:h + 1])

                    # =============== phase 6: LayerNorm epilogue ===============
                    for tb in range(NTB):
                        st = epool.tile([P, 8], F32, tag="stats")
                        sq = epool.tile([P, D], F32, tag="sq")
                        # st: 0 sum->inv, 1 sumsq, 2 mu, 3 mu^2, 4 sumsq/D,
                        #     5 var, 6 var+eps, 7 sd
                        nc.vector.tensor_tensor(st[:, 0:1], s1[tb][:, 0:1],
                                                s1[tb][:, 1:2], AL.add)
                        nc.scalar.activation(sq[:], zg[tb][:], AF.Square,
                                             accum_out=st[:, 1:2])
                        nc.vector.tensor_scalar(st[:, 2:3], st[:, 0:1], 1.0 / D,
                                                None, op0=AL.mult)
                        nc.vector.tensor_tensor(st[:, 3:4], st[:, 2:3], st[:, 2:3],
                                                AL.mult)
                        nc.vector.tensor_scalar(st[:, 4:5], st[:, 1:2], 1.0 / D,
                                                None, op0=AL.mult)
                        nc.vector.tensor_tensor(st[:, 5:6], st[:, 4:5], st[:, 3:4],
                                                AL.subtract)
                        nc.vector.tensor_scalar(st[:, 6:7], st[:, 5:6], EPS,
                                                None, op0=AL.add)
                        nc.scalar.activation(st[:, 7:8], st[:, 6:7], AF.Sqrt)
                        nc.vector.reciprocal(st[:, 0:1], st[:, 7:8])
                        fin = epool.tile([P, D], F32, tag="fin")
                        nc.vector.tensor_scalar(fin[:], zg[tb][:], st[:, 2:3],
                                                st[:, 0:1], op0=AL.subtract,
                                                op1=AL.mult)
                        nc.vector.tensor_tensor(fin[:], fin[:], sk1[tb][:], AL.add)
                        nc.sync.dma_start(out_d[tb * P:(tb + 1) * P, :], fin[:])

    return nc


# ----------------------------------------------------------------------------
# host-side input prep
# ----------------------------------------------------------------------------

def _prep_host(x, start, pre_w, pre_b, gi_w, gi_b, go_w, go_b,
               skip_w, skip_b, mix_w, mix_b, ffa_a, ffa_b):
    x = np.asarray(x, np.float32)
    start = np.asarray(start)
    a64 = np.abs(np.asarray(ffa_a, np.float64))
    b64 = np.asarray(ffa_b, np.float64)

    xc = x.reshape(NCORES, L, D)
    xT = np.ascontiguousarray(xc.transpose(0, 2, 1))          # [8, D, L]
    keep = np.ascontiguousarray(
        1.0 - start.reshape(NCORES, 1, L).astype(np.float32))

    t64 = np.arange(L, dtype=np.float64)
    c_of_p = np.arange(P) % C
    ang = b64[c_of_p][:, None] * t64[None, :]                 # [128, L]
    costab = np.cos(ang).astype(np.float32)
    nsintab = (-np.sin(ang)).astype(np.float32)

    decay = np.exp(-a64)                                      # [64]
    p_idx = np.arange(P)
    dec = np.zeros((P, 8), np.float32)
    for g in range(8):
        dec[:, g] = decay[g * 8 + p_idx // C]
    emat = np.zeros((M, 8 * P), np.float32)
    for g in range(8):
        emat[g * 8 + p_idx // C, g * P + p_idx] = 1.0

    pgw = np.ascontiguousarray(
        np.concatenate([np.asarray(pre_w, np.float32).T,
                        np.asarray(gi_w, np.float32).T], axis=1))  # [D, 128]
    pgb = np.concatenate([np.asarray(pre_b, np.float32),
                          np.asarray(gi_b, np.float32)])[:, None]  # [128,1]
    gow = np.ascontiguousarray(np.asarray(go_w, np.float32).T)
    skw = np.ascontiguousarray(np.asarray(skip_w, np.float32).T)
    ch = np.arange(M * C)
    cols_re = (ch // C) * (2 * C) + ch % C
    cols_im = cols_re + C
    mwT = np.asarray(mix_w, np.float32).T                     # [2048, 1024]
    mixw = np.ascontiguousarray(
        np.concatenate([mwT[cols_re], mwT[cols_im]], axis=0))
    ones1 = np.ones((1, P), np.float32)

    common = dict(
        costab=costab, nsintab=nsintab, dec=dec, emat=emat, ones1=ones1,
        pgw=pgw, pgb=pgb, gow=gow, skw=skw, mixw=mixw,
        gob=np.asarray(go_b, np.float32)[None, :],
        skb=np.asarray(skip_b, np.float32)[None, :],
        mixb=np.asarray(mix_b, np.float32)[None, :],
    )
    in_maps = []
    for i in range(NCORES):
        m = dict(common)
        m["xt"] = xT[i]
        m["keep"] = keep[i]
        in_maps.append(m)
    return in_maps


# ----------------------------------------------------------------------------
# host-side carry fix-up
# ----------------------------------------------------------------------------

def _fixup(out, gx_chunks, x, start, state_re, state_im,
           go_w, go_b, skip_w, skip_b, mix_w, mix_b, ffa_a, ffa_b):
    """Apply the cross-chunk carried-state correction to the prefix rows of
    each chunk (rows before the chunk's first episode reset) and compute the
    final carried state.  All in float64 on host; touches O(cores) rows for
    Bernoulli start flags."""
    a64 = np.abs(np.asarray(ffa_a, np.float64))
    b64 = np.asarray(ffa_b, np.float64)
    gamma = np.exp(-a64)[:, None] * np.exp(1j * b64)[None, :]   # [64, 16]
    log_decay = -a64[:, None]

    startc = np.asarray(start).reshape(NCORES, L)
    x64 = np.asarray(x, np.float64)
    goW = np.asarray(go_w, np.float64); goB = np.asarray(go_b, np.float64)
    skW = np.asarray(skip_w, np.float64); skB = np.asarray(skip_b, np.float64)
    mxW = np.asarray(mix_w, np.float64); mxB = np.asarray(mix_b, np.float64)

    def gamma_pow(k):
        return np.exp(log_decay * k) * np.exp(1j * b64[None, :] * k)

    # zero-carry chunk-final states S_i from device gx
    S = []
    for i in range(NCORES):
        gxi = np.asarray(gx_chunks[i], np.float64)              # [64, L]
        s_i = startc[i]
        j0 = int(np.flatnonzero(s_i)[-1]) if s_i.any() else 0
        js = np.arange(j0, L)
        expo = (L - 1 - js)
        E1 = np.exp(log_decay * expo[None, :])                  # [64, nj]
        E2 = np.exp(1j * b64[:, None] * expo[None, :])          # [16, nj]
        S.append(np.einsum('mj,mj,cj->mc', E1, gxi[:, js], E2))
    # carry chain
    Cs = [np.asarray(state_re, np.float64)[0] +
          1j * np.asarray(state_im, np.float64)[0]]             # [64,16]
    for i in range(NCORES):
        if startc[i].any():
            Cs.append(S[i])
        else:
            Cs.append(gamma_pow(L) * Cs[i] + S[i])

    # recompute prefix rows with the carried state
    for i in range(NCORES):
        nz = np.flatnonzero(startc[i])
        Pfx = int(nz[0]) if nz.size else L
        if Pfx == 0 or not np.any(np.abs(Cs[i]) > 0):
            continue
        gxi = np.asarray(gx_chunks[i], np.float64)
        s_t = Cs[i].copy()
        for t in range(Pfx):
            s_t = gamma * s_t + gxi[:, t][:, None]
            zrow = np.concatenate([s_t.real, s_t.imag], axis=1).reshape(-1)
            z = mxW @ zrow + mxB
            xr = x64[i * L + t]
            gate = 1.0 / (1.0 + np.exp(-(goW @ xr + goB)))
            skip = skW @ xr + skB
            zg = z * gate
            mu = zg.mean()
            var = ((zg - mu) ** 2).mean()
            ln = (zg - mu) / np.sqrt(var + EPS)
            out[i * L + t] = (ln + skip * (1.0 - gate)).astype(np.float32)

    final = Cs[NCORES].astype(np.complex64)[None, :, :]         # [1, 64, 16]
    return out, final


# ----------------------------------------------------------------------------
# runner (jitted shard_map over the bass_exec custom call), cached
# ----------------------------------------------------------------------------
_cache = {}


def _get_runner():
    if "fn" in _cache:
        return _cache["fn"], _cache["meta"]
    _install_patches()
    import jax
    from jax.sharding import Mesh, PartitionSpec
    from jax.experimental.shard_map import shard_map
    from concourse.bass2jax import (_bass_exec_p, partition_id_tensor,
                                    install_neuronx_cc_hook)
    install_neuronx_cc_hook()

    nc = build_program()

    in_names, out_names, out_avals = [], [], []
    partition_name = nc.partition_id_tensor.name if nc.partition_id_tensor else None
    for alloc in nc.m.functions[0].allocations:
        if not isinstance(alloc, mybir.MemoryLocationSet):
            continue
        name = alloc.memorylocations[0].name
        if alloc.kind == "ExternalInput":
            if name != partition_name:
                in_names.append(name)
        elif alloc.kind == "ExternalOutput":
            out_names.append(name)
            out_avals.append(jax.core.ShapedArray(
                tuple(alloc.tensor_shape), mybir.dt.np(alloc.dtype)))
    n_params = len(in_names)
    all_in = tuple(in_names + out_names +
                   ([partition_name] if partition_name else []))

    def _body(*args):
        operands = list(args)
        if partition_name is not None:
            operands.append(partition_id_tensor())
        outs = _bass_exec_p.bind(
            *operands,
            out_avals=tuple(out_avals),
            in_names=all_in,
            out_names=tuple(out_names),
            lowering_input_output_aliases=(),
            sim_require_finite=True,
            sim_require_nnan=True,
            nc=nc,
        )
        return tuple(outs)

    devices = jax.devices()[:NCORES]
    mesh = Mesh(np.asarray(devices), ("core",))
    nin = n_params + len(out_names)
    fn = jax.jit(shard_map(_body, mesh=mesh,
                           in_specs=(PartitionSpec("core"),) * nin,
                           out_specs=(PartitionSpec("core"),) * len(out_names),
                           check_rep=False))
    meta = (in_names, out_names, out_avals)
    _cache["fn"] = fn
    _cache["meta"] = meta
    return fn, meta


def run_device(in_maps):
    import jax
    fn, (in_names, out_names, out_avals) = _get_runner()
    concat_in = [
        np.concatenate([np.asarray(in_maps[c][name]) for c in range(NCORES)],
                       axis=0)
        for name in in_names
    ]
    concat_zeros = [
        np.zeros((NCORES * a.shape[0], *a.shape[1:]), a.dtype)
        for a in out_avals
    ]
    outs = fn(*concat_in, *concat_zeros)
    jax.block_until_ready(outs)
    res = {}
    for i, name in enumerate(out_names):
        res[name] = np.asarray(outs[i]).reshape(NCORES, *out_avals[i].shape)
    return res


# ----------------------------------------------------------------------------
# public entry point
# ----------------------------------------------------------------------------

def kernel(x, state_re, state_im, start, next_done,
           pre_w, pre_b, gi_w, gi_b, go_w, go_b,
           skip_w, skip_b, mix_w, mix_b, ffa_a, ffa_b):
    in_maps = _prep_host(x, start, pre_w, pre_b, gi_w, gi_b, go_w, go_b,
                         skip_w, skip_b, mix_w, mix_b, ffa_a, ffa_b)
    res = run_device(in_maps)
    out = res["out"].reshape(T, D).astype(np.float32).copy()
    gx_chunks = [res["gx"][i] for i in range(NCORES)]
    out, final = _fixup(out, gx_chunks, x, start, state_re, state_im,
                        go_w, go_b, skip_w, skip_b, mix_w, mix_b,
                        ffa_a, ffa_b)
    return out, final


# revision 10
# speedup vs baseline: 1.3633x; 1.3633x over previous
"""Trainium2 Bass kernel for nn_FFM_71347996721190.

Model: gated complex linear recurrence (FFM) + dense in/out projections +
gated LayerNorm residual.  T=8192 timesteps are sharded across 8 NeuronCores
(sequence parallel, 1024 steps per core); each core runs the full pipeline on
its chunk with zero carried-in state, and the tiny cross-chunk carry fix-up
(prefix rows before each chunk's first episode reset) is applied on the host.

Device algorithm per core (chunk length L=1024):
  gate_in/pre      : f32r matmuls  x @ {gi,pre}_w.T (packed in one PSUM)
  gated_x          : sigmoid + multiply   -> gxT [64, L]
  scan             : rotating-frame trick.  s_t = gamma*s_{t-1}+x_t with
                     gamma = exp(-|a| + i*b) decouples into two REAL scans
                     u_t = e^{-a} u_{t-1} + cos(b t) x_t  (and -sin for imag)
                     via the DVE tensor_tensor_scan instruction, with episode
                     resets folded into the per-step multiplier A_t
                     (A_t = keep_t * e^{-a}).  Recover s = e^{i b t} u with
                     elementwise cos/sin tables (host-precomputed in f64).
                     Channel layout: partition p of group g <-> trace
                     m = 8g + p//16, context c = p%16  (8 groups x 128).
  z                : f32r matmul  [Re(s);Im(s)] (channel-major, stationary)
                     @ mix_w.T (moving), accumulated over 16 k-chunks
  gate_out/skip    : f32r matmuls with x^T tiles stationary
  epilogue         : zg = z*sigmoid(go); LayerNorm(zg); + skip*(1-sigmoid(go))

Everything is emitted through TileContext (auto-sync).  Two environment
workarounds are inlined: the walrus build here rejects >1 semaphore wait per
instruction (waits are hoisted onto single-wait NoOps in a BIR post-pass) and
the TileContext tail drain carries many waits (same fix at emission time).
"""

import contextlib
import json
import numpy as np

import bass_rust
import concourse.bass as bass
import concourse.mybir as mybir
from concourse import tile
from concourse import bass2jax

# ----------------------------------------------------------------------------
# problem constants (hardcoded per harness contract)
# ----------------------------------------------------------------------------
T = 8192
D = 1024          # INPUT == OUT
M = 64            # trace
C = 16            # context
NCORES = 8
L = T // NCORES   # 1024 timesteps per core
P = 128
EPS = 1e-5
F32 = mybir.dt.float32
F32R = mybir.dt.float32r
AL = mybir.AluOpType
AF = mybir.ActivationFunctionType

NG = 8            # scan partition groups (8 x 128 = 1024 (m,c) channels)
NTB = L // P      # 8 time blocks per chunk
KD = D // P       # 8 k-chunks over INPUT
KCH = 16          # k-chunks over 2*M*C for mix

# ----------------------------------------------------------------------------
# environment workarounds
# ----------------------------------------------------------------------------
_patched = False


def _patched_drain_and_barrier(self, tick_clock, wait_clock):
    # stock version puts every outstanding sem wait on one Drain; this
    # walrus build allows at most one sync wait per instruction.
    nc = self.nc
    vals = list(tick_clock.global_clock)
    nc.sync.drain()
    for i, v in enumerate(vals):
        if v:
            single = [0] * len(vals)
            single[i] = v
            n = nc.sync.nop(nofuse=True)
            wait_clock.add_sem_waits(
                n.ins, tile.ScopedClock({None: bass_rust.VectorClock(single)})
            )
    nc.all_engine_barrier()
    assert self.sems is not None
    popped = nc._tile_sem_poison_stack.pop()
    assert popped is self._sem_poison
    nc.clear_and_free_semaphores(list(self.sems.allocated().values()))
    nc.all_engine_barrier()


_wsplit_ctr = [0]


def _fix_bir_json(d):
    """Hoist multi-waits (and all waits on PE compute instructions, whose
    ldweights lowering slot takes none) onto dedicated single-wait NoOps."""
    for fn in d.get("functions", []):
        for blk in fn.get("blocks", []):
            out = []
            for inst in blk.get("instructions", []):
                si = inst.get("sync_info")
                waits = (si or {}).get("on_wait") or []
                engine = inst.get("engine")
                is_pe_compute = engine == "PE" and inst.get("opcode") not in (
                    "NoOp", "Drain", "EventSemaphore")
                if len(waits) > 1 or (is_pe_compute and len(waits) >= 1):
                    for w in waits:
                        _wsplit_ctr[0] += 1
                        nop = {
                            "engine": engine,
                            "ins": [],
                            "outs": [],
                            "name": f"I-wsplit-{_wsplit_ctr[0]}",
                            "opcode": "NoOp",
                            "sync_info": {"on_wait": [w], "on_update": []},
                        }
                        if "debug" in inst:
                            nop["debug"] = inst["debug"]
                        out.append(nop)
                    si["on_wait"] = []
                out.append(inst)
            blk["instructions"] = out


def _install_patches():
    global _patched
    if _patched:
        return
    tile.TileContext._drain_and_barrier = _patched_drain_and_barrier

    orig = bass2jax.compile_bir_kernel

    def wrapped(ant_bir_str, compile_dir_path, neff_name=None, **kw):
        d = json.loads(ant_bir_str)
        _fix_bir_json(d)
        return orig(json.dumps(d).encode(), compile_dir_path,
                    neff_name=neff_name, **kw)

    bass2jax.compile_bir_kernel = wrapped
    _patched = True


# ----------------------------------------------------------------------------
# bass program (identical for all 8 cores)
# ----------------------------------------------------------------------------

def build_program():
    nc = bass.Bass()

    xt_d = nc.dram_tensor("xt", [D, L], F32R, kind="ExternalInput")
    keep_d = nc.dram_tensor("keep", [1, L], F32R, kind="ExternalInput")
    costab_d = nc.dram_tensor("costab", [P, L], F32, kind="ExternalInput")
    nsintab_d = nc.dram_tensor("nsintab", [P, L], F32, kind="ExternalInput")
    dec_d = nc.dram_tensor("dec", [P, 8], F32, kind="ExternalInput")
    emat_d = nc.dram_tensor("emat", [M, 8 * P], F32R, kind="ExternalInput")
    ones1_d = nc.dram_tensor("ones1", [1, P], F32R, kind="ExternalInput")
    pgw_d = nc.dram_tensor("pgw", [D, P], F32R, kind="ExternalInput")
    pgb_d = nc.dram_tensor("pgb", [P, 1], F32, kind="ExternalInput")
    gow_d = nc.dram_tensor("gow", [D, D], F32R, kind="ExternalInput")
    skw_d = nc.dram_tensor("skw", [D, D], F32R, kind="ExternalInput")
    mixw_d = nc.dram_tensor("mixw", [2 * M * C, D], F32R, kind="ExternalInput")
    gob_d = nc.dram_tensor("gob", [1, D], F32R, kind="ExternalInput")
    skb_d = nc.dram_tensor("skb", [1, D], F32R, kind="ExternalInput")
    mixb_d = nc.dram_tensor("mixb", [1, D], F32R, kind="ExternalInput")

    out_d = nc.dram_tensor("out", [L, D], F32, kind="ExternalOutput")
    gx_d = nc.dram_tensor("gx", [M, L], F32R, kind="ExternalOutput")

    with tile.TileContext(nc) as tc:
        with (
            tc.tile_pool(name="const", bufs=1) as cpool,
            tc.tile_pool(name="s", bufs=1) as spool,
        ):
            xt_stack = contextlib.ExitStack()
            xpool = xt_stack.enter_context(
                tc.tile_pool(name="xtp", bufs=1, side="right"))
            # --- long-lived tiles ---
            ones1 = cpool.tile([1, P], F32R, tag="ones1")
            gob_t = cpool.tile([1, D], F32R, tag="gob")
            skb_t = cpool.tile([1, D], F32R, tag="skb")
            mixb_t = cpool.tile([1, D], F32R, tag="mixb")
            gx = cpool.tile([M, L], F32R, tag="gxs")
            nc.sync.dma_start(ones1[:], ones1_d[:])
            nc.sync.dma_start(gob_t[:], gob_d[:])
            nc.sync.dma_start(skb_t[:], skb_d[:])
            nc.sync.dma_start(mixb_t[:], mixb_d[:])

            s_re = [spool.tile([P, L], F32R, tag=f"sre{g}", name=f"sre{g}")
                    for g in range(NG)]
            s_im = [spool.tile([P, L], F32R, tag=f"sim{g}", name=f"sim{g}")
                    for g in range(NG)]

            xt = xpool.tile([P, KD * L], F32R, tag="xt")
            for kc in range(KD):
                nc.sync.dma_start(xt[:, kc * L:(kc + 1) * L],
                                  xt_d[kc * P:(kc + 1) * P, :])

            # =============== phase 1-3: scan path ===============
            with (
                tc.tile_pool(name="scanc", bufs=1, side="right") as scpool,
                tc.tile_pool(name="psA", bufs=1, space="PSUM") as psA,
                tc.tile_pool(name="psR", bufs=2, space="PSUM") as psR,
                tc.tile_pool(name="scant", bufs=2, side="right") as tpool,
                tc.tile_pool(name="u", bufs=2, side="right") as upool,
                tc.tile_pool(name="p1", bufs=1, side="right") as p1pool,
            ):
                costab = scpool.tile([P, L], F32, tag="costab")
                nsintab = scpool.tile([P, L], F32, tag="nsintab")
                dec = scpool.tile([P, 8], F32, tag="dec")
                emat = scpool.tile([M, 8 * P], F32R, tag="emat")
                pgw = scpool.tile([P, D], F32R, tag="pgw")
                pgb = scpool.tile([P, 1], F32, tag="pgb")
                keep = scpool.tile([1, L], F32R, tag="keep")
                keeprep = scpool.tile([P, L], F32, tag="keeprep")
                nc.sync.dma_start(costab[:], costab_d[:])
                nc.sync.dma_start(nsintab[:], nsintab_d[:])
                nc.sync.dma_start(dec[:], dec_d[:])
                nc.sync.dma_start(emat[:], emat_d[:])
                nc.sync.dma_start(pgb[:], pgb_d[:])
                nc.sync.dma_start(keep[:], keep_d[:])
                for kc in range(KD):
                    nc.sync.dma_start(pgw[:, kc * P:(kc + 1) * P],
                                      pgw_d[kc * P:(kc + 1) * P, :])

                pgps = psA.tile([P, L], F32, tag="pg")
                for th in range(2):
                    sl = slice(th * 512, (th + 1) * 512)
                    for kc in range(KD):
                        nc.tensor.matmul(
                            pgps[:, sl],
                            lhsT=pgw[:, kc * P:(kc + 1) * P],
                            rhs=xt[:, kc * L + th * 512: kc * L + (th + 1) * 512],
                            start=(kc == 0), stop=(kc == KD - 1))
                pg = p1pool.tile([P, L], F32, tag="pg1")
                gi2 = p1pool.tile([M, L], F32, tag="gi2")
                # pre rows 0:64 (+bias), sigmoid(gi) rows 64:128 (+bias)
                nc.scalar.activation(pg[0:M, :], pgps[0:M, :], AF.Identity,
                                     bias=pgb[0:M, 0:1])
                nc.scalar.activation(pg[M:P, :], pgps[M:P, :], AF.Sigmoid,
                                     bias=pgb[M:P, 0:1])
                nc.sync.dma_start(gi2[:], pg[M:P, :])
                nc.vector.tensor_tensor(gx[:], pg[0:M, :], gi2[:], AL.mult)
                nc.sync.dma_start(gx_d[:], gx[:])

                # keep replication to all 128 partitions
                kps = psA.tile([P, L], F32, tag="keep")
                for th in range(2):
                    nc.tensor.matmul(kps[:, th * 512:(th + 1) * 512],
                                     lhsT=ones1[:],
                                     rhs=keep[:, th * 512:(th + 1) * 512],
                                     start=True, stop=True)
                nc.scalar.copy(keeprep[:], kps[:])

                # per group: replicate gx, scan, recover s
                for g in range(NG):
                    rps = psR.tile([P, L], F32, tag="repl")
                    for th in range(2):
                        nc.tensor.matmul(
                            rps[:, th * 512:(th + 1) * 512],
                            lhsT=emat[:, g * P:(g + 1) * P],
                            rhs=gx[:, th * 512:(th + 1) * 512],
                            start=True, stop=True)
                    b_re = tpool.tile([P, L], F32, tag="bre")
                    b_im = tpool.tile([P, L], F32, tag="bim")
                    a_g = tpool.tile([P, L], F32, tag="ag")
                    nc.vector.tensor_tensor(b_re[:], rps[:], costab[:], AL.mult)
                    nc.vector.tensor_tensor(b_im[:], rps[:], nsintab[:], AL.mult)
                    nc.vector.tensor_scalar(a_g[:], keeprep[:], dec[:, g:g + 1],
                                            None, op0=AL.mult)
                    u_re = upool.tile([P, L], F32, tag="ure")
                    u_im = upool.tile([P, L], F32, tag="uim")
                    nc.vector.tensor_tensor_scan(u_re[:], a_g[:], b_re[:], 0.0,
                                                 op0=AL.mult, op1=AL.add)
                    nc.vector.tensor_tensor_scan(u_im[:], a_g[:], b_im[:], 0.0,
                                                 op0=AL.mult, op1=AL.add)
                    # s_re = cos*u_re + (-sin)*u_im
                    # s_im = cos*u_im - (-sin)*u_re
                    t1 = tpool.tile([P, L], F32, tag="t1")
                    t2 = tpool.tile([P, L], F32, tag="t2")
                    nc.vector.tensor_tensor(t1[:], costab[:], u_re[:], AL.mult)
                    nc.vector.tensor_tensor(t2[:], nsintab[:], u_im[:], AL.mult)
                    nc.vector.tensor_tensor(s_re[g][:], t1[:], t2[:], AL.add)
                    nc.vector.tensor_tensor(t1[:], costab[:], u_im[:], AL.mult)
                    nc.vector.tensor_tensor(t2[:], nsintab[:], u_re[:], AL.mult)
                    nc.vector.tensor_tensor(s_im[g][:], t1[:], t2[:], AL.subtract)

            # =============== phase 4: gate_out / skip ===============
            with tc.tile_pool(name="gos", bufs=1) as gpool:
              go_sig = [gpool.tile([P, D], F32, tag=f"gosig{tb}",
                                   name=f"gosig{tb}") for tb in range(NTB)]
              sk1 = [gpool.tile([P, D], F32, tag=f"sk1{tb}",
                                name=f"sk1{tb}") for tb in range(NTB)]
              with (
                tc.tile_pool(name="psB4", bufs=1, space="PSUM") as psB4,
                tc.tile_pool(name="w4", bufs=2, side="right") as wpool4,
                tc.tile_pool(name="gneg", bufs=2, side="right") as npool,
              ):
                # four passes (go half0, go half1, sk half0, sk half1);
                # weights loaded once per (pass, kc); 8 PSUM banks = one per tb
                for pi, (is_go, h) in enumerate(
                        [(True, 0), (True, 1), (False, 0), (False, 1)]):
                    hs = slice(h * 512, (h + 1) * 512)
                    wdram = gow_d if is_go else skw_d
                    brow = gob_t if is_go else skb_t
                    zb = [psB4.tile([P, 512], F32, tag=f"bank{i}",
                                    name=f"b4_{pi}_{i}") for i in range(NTB)]
                    for tb in range(NTB):
                        nc.tensor.matmul(zb[tb][:], lhsT=ones1[:],
                                         rhs=brow[:, hs],
                                         start=True, stop=False)
                    for kc in range(KD):
                        w = wpool4.tile([P, 512], F32R, tag="w4", bufs=3,
                                        name=f"w4_{pi}_{kc}")
                        nc.sync.dma_start(w[:], wdram[kc * P:(kc + 1) * P, hs])
                        for tb in range(NTB):
                            nc.tensor.matmul(
                                zb[tb][:],
                                lhsT=xt[:, kc * L + tb * P: kc * L + (tb + 1) * P],
                                rhs=w[:],
                                start=False, stop=(kc == KD - 1))
                    for tb in range(NTB):
                        if is_go:
                            nc.scalar.activation(go_sig[tb][:, hs], zb[tb][:],
                                                 AF.Sigmoid)
                        else:
                            gneg = npool.tile([P, 512], F32, tag="gneg",
                                              name=f"gneg_{pi}_{tb}")
                            nc.vector.tensor_scalar(gneg[:], go_sig[tb][:, hs],
                                                    -1.0, 1.0, op0=AL.mult,
                                                    op1=AL.add)
                            nc.vector.scalar_tensor_tensor(
                                sk1[tb][:, hs], zb[tb][:], 0.0, gneg[:],
                                op0=AL.bypass, op1=AL.mult)

              # xt no longer needed; free its SBUF before the mix phase
              xt_stack.close()
              # =============== phase 5: mix matmul + zg ===============
              if True:
                with (
                    tc.tile_pool(name="psB5", bufs=1, space="PSUM") as psB5,
                    tc.tile_pool(name="w5", bufs=4) as wpool5,
                    tc.tile_pool(name="zgp", bufs=1) as zpool,
                    tc.tile_pool(name="ep", bufs=2) as epool,
                ):
                    zg = [zpool.tile([P, D], F32, tag=f"zg{tb}", name=f"zg{tb}")
                          for tb in range(NTB)]
                    s1 = [zpool.tile([P, 2], F32, tag=f"s1_{tb}", name=f"s1_{tb}")
                          for tb in range(NTB)]
                    for h in range(2):
                        hs = slice(h * 512, (h + 1) * 512)
                        zb = [psB5.tile([P, 512], F32, tag=f"bank{i}",
                                        name=f"bank5_{h}_{i}") for i in range(NTB)]
                        for tb in range(NTB):
                            nc.tensor.matmul(
                                zb[tb][:], lhsT=ones1[:],
                                rhs=mixb_t[:, h * 512:(h + 1) * 512],
                                start=True, stop=False)
                        for cc in range(KCH):
                            mw = wpool5.tile([P, 512], F32R, tag="w5")
                            nc.sync.dma_start(mw[:], mixw_d[cc * P:(cc + 1) * P, hs])
                            stile = s_re[cc] if cc < NG else s_im[cc - NG]
                            for tb in range(NTB):
                                nc.tensor.matmul(
                                    zb[tb][:],
                                    lhsT=stile[:, tb * P:(tb + 1) * P],
                                    rhs=mw[:],
                                    start=False, stop=(cc == KCH - 1))
                        for tb in range(NTB):
                            nc.vector.scalar_tensor_tensor(
                                zg[tb][:, hs], zb[tb][:], 0.0, go_sig[tb][:, hs],
                                op0=AL.bypass, op1=AL.mult,
                                accum_out=s1[tb][:, h:h + 1])

                    # =============== phase 6: LayerNorm epilogue ===============
                    for tb in range(NTB):
                        st = epool.tile([P, 8], F32, tag="stats")
                        sq = epool.tile([P, D], F32, tag="sq")
                        # st: 0 sum->inv, 1 sumsq, 2 mu, 3 mu^2, 4 sumsq/D,
                        #     5 var, 6 var+eps, 7 sd
                        nc.vector.tensor_tensor(st[:, 0:1], s1[tb][:, 0:1],
                                                s1[tb][:, 1:2], AL.add)
                        nc.scalar.activation(sq[:], zg[tb][:], AF.Square,
                                             accum_out=st[:, 1:2])
                        nc.vector.tensor_scalar(st[:, 2:3], st[:, 0:1], 1.0 / D,
                                                None, op0=AL.mult)
                        nc.vector.tensor_tensor(st[:, 3:4], st[:, 2:3], st[:, 2:3],
                                                AL.mult)
                        nc.vector.tensor_scalar(st[:, 4:5], st[:, 1:2], 1.0 / D,
                                                None, op0=AL.mult)
                        nc.vector.tensor_tensor(st[:, 5:6], st[:, 4:5], st[:, 3:4],
                                                AL.subtract)
                        nc.vector.tensor_scalar(st[:, 6:7], st[:, 5:6], EPS,
                                                None, op0=AL.add)
                        nc.scalar.activation(st[:, 7:8], st[:, 6:7], AF.Sqrt)
                        nc.vector.reciprocal(st[:, 0:1], st[:, 7:8])
                        fin = epool.tile([P, D], F32, tag="fin")
                        nc.vector.tensor_scalar(fin[:], zg[tb][:], st[:, 2:3],
                                                st[:, 0:1], op0=AL.subtract,
                                                op1=AL.mult)
                        nc.vector.tensor_tensor(fin[:], fin[:], sk1[tb][:], AL.add)
                        nc.sync.dma_start(out_d[tb * P:(tb + 1) * P, :], fin[:])

    return nc


# ----------------------------------------------------------------------------
# host-side input prep
# ----------------------------------------------------------------------------

def _prep_host(x, start, pre_w, pre_b, gi_w, gi_b, go_w, go_b,
               skip_w, skip_b, mix_w, mix_b, ffa_a, ffa_b):
    x = np.asarray(x, np.float32)
    start = np.asarray(start)
    a64 = np.abs(np.asarray(ffa_a, np.float64))
    b64 = np.asarray(ffa_b, np.float64)

    xc = x.reshape(NCORES, L, D)
    xT = np.ascontiguousarray(xc.transpose(0, 2, 1))          # [8, D, L]
    keep = np.ascontiguousarray(
        1.0 - start.reshape(NCORES, 1, L).astype(np.float32))

    t64 = np.arange(L, dtype=np.float64)
    c_of_p = np.arange(P) % C
    ang = b64[c_of_p][:, None] * t64[None, :]                 # [128, L]
    costab = np.cos(ang).astype(np.float32)
    nsintab = (-np.sin(ang)).astype(np.float32)

    decay = np.exp(-a64)                                      # [64]
    p_idx = np.arange(P)
    dec = np.zeros((P, 8), np.float32)
    for g in range(8):
        dec[:, g] = decay[g * 8 + p_idx // C]
    emat = np.zeros((M, 8 * P), np.float32)
    for g in range(8):
        emat[g * 8 + p_idx // C, g * P + p_idx] = 1.0

    pgw = np.ascontiguousarray(
        np.concatenate([np.asarray(pre_w, np.float32).T,
                        np.asarray(gi_w, np.float32).T], axis=1))  # [D, 128]
    pgb = np.concatenate([np.asarray(pre_b, np.float32),
                          np.asarray(gi_b, np.float32)])[:, None]  # [128,1]
    gow = np.ascontiguousarray(np.asarray(go_w, np.float32).T)
    skw = np.ascontiguousarray(np.asarray(skip_w, np.float32).T)
    ch = np.arange(M * C)
    cols_re = (ch // C) * (2 * C) + ch % C
    cols_im = cols_re + C
    mwT = np.asarray(mix_w, np.float32).T                     # [2048, 1024]
    mixw = np.ascontiguousarray(
        np.concatenate([mwT[cols_re], mwT[cols_im]], axis=0))
    ones1 = np.ones((1, P), np.float32)

    common = dict(
        costab=costab, nsintab=nsintab, dec=dec, emat=emat, ones1=ones1,
        pgw=pgw, pgb=pgb, gow=gow, skw=skw, mixw=mixw,
        gob=np.asarray(go_b, np.float32)[None, :],
        skb=np.asarray(skip_b, np.float32)[None, :],
        mixb=np.asarray(mix_b, np.float32)[None, :],
    )
    in_maps = []
    for i in range(NCORES):
        m = dict(common)
        m["xt"] = xT[i]
        m["keep"] = keep[i]
        in_maps.append(m)
    return in_maps


# ----------------------------------------------------------------------------
# host-side carry fix-up
# ----------------------------------------------------------------------------

def _fixup(out, gx_chunks, x, start, state_re, state_im,
           go_w, go_b, skip_w, skip_b, mix_w, mix_b, ffa_a, ffa_b):
    """Apply the cross-chunk carried-state correction to the prefix rows of
    each chunk (rows before the chunk's first episode reset) and compute the
    final carried state.  All in float64 on host; touches O(cores) rows for
    Bernoulli start flags."""
    a64 = np.abs(np.asarray(ffa_a, np.float64))
    b64 = np.asarray(ffa_b, np.float64)
    gamma = np.exp(-a64)[:, None] * np.exp(1j * b64)[None, :]   # [64, 16]
    log_decay = -a64[:, None]

    startc = np.asarray(start).reshape(NCORES, L)
    x64 = np.asarray(x, np.float64)
    goW = np.asarray(go_w, np.float64); goB = np.asarray(go_b, np.float64)
    skW = np.asarray(skip_w, np.float64); skB = np.asarray(skip_b, np.float64)
    mxW = np.asarray(mix_w, np.float64); mxB = np.asarray(mix_b, np.float64)

    def gamma_pow(k):
        return np.exp(log_decay * k) * np.exp(1j * b64[None, :] * k)

    # zero-carry chunk-final states S_i from device gx
    S = []
    for i in range(NCORES):
        gxi = np.asarray(gx_chunks[i], np.float64)              # [64, L]
        s_i = startc[i]
        j0 = int(np.flatnonzero(s_i)[-1]) if s_i.any() else 0
        js = np.arange(j0, L)
        expo = (L - 1 - js)
        E1 = np.exp(log_decay * expo[None, :])                  # [64, nj]
        E2 = np.exp(1j * b64[:, None] * expo[None, :])          # [16, nj]
        S.append(np.einsum('mj,mj,cj->mc', E1, gxi[:, js], E2))
    # carry chain
    Cs = [np.asarray(state_re, np.float64)[0] +
          1j * np.asarray(state_im, np.float64)[0]]             # [64,16]
    for i in range(NCORES):
        if startc[i].any():
            Cs.append(S[i])
        else:
            Cs.append(gamma_pow(L) * Cs[i] + S[i])

    # recompute prefix rows with the carried state
    for i in range(NCORES):
        nz = np.flatnonzero(startc[i])
        Pfx = int(nz[0]) if nz.size else L
        if Pfx == 0 or not np.any(np.abs(Cs[i]) > 0):
            continue
        gxi = np.asarray(gx_chunks[i], np.float64)
        s_t = Cs[i].copy()
        for t in range(Pfx):
            s_t = gamma * s_t + gxi[:, t][:, None]
            zrow = np.concatenate([s_t.real, s_t.imag], axis=1).reshape(-1)
            z = mxW @ zrow + mxB
            xr = x64[i * L + t]
            gate = 1.0 / (1.0 + np.exp(-(goW @ xr + goB)))
            skip = skW @ xr + skB
            zg = z * gate
            mu = zg.mean()
            var = ((zg - mu) ** 2).mean()
            ln = (zg - mu) / np.sqrt(var + EPS)
            out[i * L + t] = (ln + skip * (1.0 - gate)).astype(np.float32)

    final = Cs[NCORES].astype(np.complex64)[None, :, :]         # [1, 64, 16]
    return out, final


# ----------------------------------------------------------------------------
# runner (jitted shard_map over the bass_exec custom call), cached
# ----------------------------------------------------------------------------
_cache = {}


def _get_runner():
    if "fn" in _cache:
        return _cache["fn"], _cache["meta"]
    _install_patches()
    import jax
    from jax.sharding import Mesh, PartitionSpec
    from jax.experimental.shard_map import shard_map
    from concourse.bass2jax import (_bass_exec_p, partition_id_tensor,
                                    install_neuronx_cc_hook)
    install_neuronx_cc_hook()

    nc = build_program()

    in_names, out_names, out_avals = [], [], []
    partition_name = nc.partition_id_tensor.name if nc.partition_id_tensor else None
    for alloc in nc.m.functions[0].allocations:
        if not isinstance(alloc, mybir.MemoryLocationSet):
            continue
        name = alloc.memorylocations[0].name
        if alloc.kind == "ExternalInput":
            if name != partition_name:
                in_names.append(name)
        elif alloc.kind == "ExternalOutput":
            out_names.append(name)
            out_avals.append(jax.core.ShapedArray(
                tuple(alloc.tensor_shape), mybir.dt.np(alloc.dtype)))
    n_params = len(in_names)
    all_in = tuple(in_names + out_names +
                   ([partition_name] if partition_name else []))

    def _body(*args):
        operands = list(args)
        if partition_name is not None:
            operands.append(partition_id_tensor())
        outs = _bass_exec_p.bind(
            *operands,
            out_avals=tuple(out_avals),
            in_names=all_in,
            out_names=tuple(out_names),
            lowering_input_output_aliases=(),
            sim_require_finite=True,
            sim_require_nnan=True,
            nc=nc,
        )
        return tuple(outs)

    devices = jax.devices()[:NCORES]
    mesh = Mesh(np.asarray(devices), ("core",))
    nin = n_params + len(out_names)
    fn = jax.jit(shard_map(_body, mesh=mesh,
                           in_specs=(PartitionSpec("core"),) * nin,
                           out_specs=(PartitionSpec("core"),) * len(out_names),
                           check_rep=False))
    meta = (in_names, out_names, out_avals)
    _cache["fn"] = fn
    _cache["meta"] = meta
    return fn, meta


def run_device(in_maps):
    import jax
    fn, (in_names, out_names, out_avals) = _get_runner()
    concat_in = [
        np.concatenate([np.asarray(in_maps[c][name]) for c in range(NCORES)],
                       axis=0)
        for name in in_names
    ]
    concat_zeros = [
        np.zeros((NCORES * a.shape[0], *a.shape[1:]), a.dtype)
        for a in out_avals
    ]
    outs = fn(*concat_in, *concat_zeros)
    jax.block_until_ready(outs)
    res = {}
    for i, name in enumerate(out_names):
        res[name] = np.asarray(outs[i]).reshape(NCORES, *out_avals[i].shape)
    return res


# ----------------------------------------------------------------------------
# public entry point
# ----------------------------------------------------------------------------

def kernel(x, state_re, state_im, start, next_done,
           pre_w, pre_b, gi_w, gi_b, go_w, go_b,
           skip_w, skip_b, mix_w, mix_b, ffa_a, ffa_b):
    in_maps = _prep_host(x, start, pre_w, pre_b, gi_w, gi_b, go_w, go_b,
                         skip_w, skip_b, mix_w, mix_b, ffa_a, ffa_b)
    res = run_device(in_maps)
    out = res["out"].reshape(T, D).astype(np.float32).copy()
    gx_chunks = [res["gx"][i] for i in range(NCORES)]
    out, final = _fixup(out, gx_chunks, x, start, state_re, state_im,
                        go_w, go_b, skip_w, skip_b, mix_w, mix_b,
                        ffa_a, ffa_b)
    return out, final


# revision 11
# speedup vs baseline: 1.4076x; 1.0324x over previous
"""Trainium2 Bass kernel for nn_FFM_71347996721190.

Model: gated complex linear recurrence (FFM) + dense in/out projections +
gated LayerNorm residual.  T=8192 timesteps are sharded across 8 NeuronCores
(sequence parallel, 1024 steps per core); each core runs the full pipeline on
its chunk with zero carried-in state, and the tiny cross-chunk carry fix-up
(prefix rows before each chunk's first episode reset) is applied on the host.

Device algorithm per core (chunk length L=1024):
  gate_in/pre      : f32r matmuls  x @ {gi,pre}_w.T (packed in one PSUM)
  gated_x          : sigmoid + multiply   -> gxT [64, L]
  scan             : rotating-frame trick.  s_t = gamma*s_{t-1}+x_t with
                     gamma = exp(-|a| + i*b) decouples into two REAL scans
                     u_t = e^{-a} u_{t-1} + cos(b t) x_t  (and -sin for imag)
                     via the DVE tensor_tensor_scan instruction, with episode
                     resets folded into the per-step multiplier A_t
                     (A_t = keep_t * e^{-a}).  Recover s = e^{i b t} u with
                     elementwise cos/sin tables (host-precomputed in f64).
                     Channel layout: partition p of group g <-> trace
                     m = 8g + p//16, context c = p%16  (8 groups x 128).
  z                : f32r matmul  [Re(s);Im(s)] (channel-major, stationary)
                     @ mix_w.T (moving), accumulated over 16 k-chunks
  gate_out/skip    : f32r matmuls with x^T tiles stationary
  epilogue         : zg = z*sigmoid(go); LayerNorm(zg); + skip*(1-sigmoid(go))

Everything is emitted through TileContext (auto-sync).  Two environment
workarounds are inlined: the walrus build here rejects >1 semaphore wait per
instruction (waits are hoisted onto single-wait NoOps in a BIR post-pass) and
the TileContext tail drain carries many waits (same fix at emission time).
"""

import contextlib
import json
import numpy as np

import bass_rust
import concourse.bass as bass
import concourse.mybir as mybir
from concourse import tile
from concourse import bass2jax

# ----------------------------------------------------------------------------
# problem constants (hardcoded per harness contract)
# ----------------------------------------------------------------------------
T = 8192
D = 1024          # INPUT == OUT
M = 64            # trace
C = 16            # context
NCORES = 8
L = T // NCORES   # 1024 timesteps per core
P = 128
EPS = 1e-5
F32 = mybir.dt.float32
F32R = mybir.dt.float32r
AL = mybir.AluOpType
AF = mybir.ActivationFunctionType

NG = 8            # scan partition groups (8 x 128 = 1024 (m,c) channels)
NTB = L // P      # 8 time blocks per chunk
KD = D // P       # 8 k-chunks over INPUT
KCH = 16          # k-chunks over 2*M*C for mix

# ----------------------------------------------------------------------------
# environment workarounds
# ----------------------------------------------------------------------------
_patched = False


def _patched_drain_and_barrier(self, tick_clock, wait_clock):
    # stock version puts every outstanding sem wait on one Drain; this
    # walrus build allows at most one sync wait per instruction.
    nc = self.nc
    vals = list(tick_clock.global_clock)
    nc.sync.drain()
    for i, v in enumerate(vals):
        if v:
            single = [0] * len(vals)
            single[i] = v
            n = nc.sync.nop(nofuse=True)
            wait_clock.add_sem_waits(
                n.ins, tile.ScopedClock({None: bass_rust.VectorClock(single)})
            )
    nc.all_engine_barrier()
    assert self.sems is not None
    popped = nc._tile_sem_poison_stack.pop()
    assert popped is self._sem_poison
    nc.clear_and_free_semaphores(list(self.sems.allocated().values()))
    nc.all_engine_barrier()


_wsplit_ctr = [0]


def _fix_bir_json(d):
    """Hoist multi-waits (and all waits on PE compute instructions, whose
    ldweights lowering slot takes none) onto dedicated single-wait NoOps."""
    for fn in d.get("functions", []):
        for blk in fn.get("blocks", []):
            out = []
            for inst in blk.get("instructions", []):
                si = inst.get("sync_info")
                waits = (si or {}).get("on_wait") or []
                engine = inst.get("engine")
                is_pe_compute = engine == "PE" and inst.get("opcode") not in (
                    "NoOp", "Drain", "EventSemaphore")
                if len(waits) > 1 or (is_pe_compute and len(waits) >= 1):
                    for w in waits:
                        _wsplit_ctr[0] += 1
                        nop = {
                            "engine": engine,
                            "ins": [],
                            "outs": [],
                            "name": f"I-wsplit-{_wsplit_ctr[0]}",
                            "opcode": "NoOp",
                            "sync_info": {"on_wait": [w], "on_update": []},
                        }
                        if "debug" in inst:
                            nop["debug"] = inst["debug"]
                        out.append(nop)
                    si["on_wait"] = []
                out.append(inst)
            blk["instructions"] = out


def _install_patches():
    global _patched
    if _patched:
        return
    tile.TileContext._drain_and_barrier = _patched_drain_and_barrier

    orig = bass2jax.compile_bir_kernel

    def wrapped(ant_bir_str, compile_dir_path, neff_name=None, **kw):
        d = json.loads(ant_bir_str)
        _fix_bir_json(d)
        return orig(json.dumps(d).encode(), compile_dir_path,
                    neff_name=neff_name, **kw)

    bass2jax.compile_bir_kernel = wrapped
    _patched = True


# ----------------------------------------------------------------------------
# bass program (identical for all 8 cores)
# ----------------------------------------------------------------------------

def build_program():
    nc = bass.Bass()

    xt_d = nc.dram_tensor("xt", [D, L], F32R, kind="ExternalInput")
    keep_d = nc.dram_tensor("keep", [1, L], F32R, kind="ExternalInput")
    costab_d = nc.dram_tensor("costab", [P, L], F32, kind="ExternalInput")
    nsintab_d = nc.dram_tensor("nsintab", [P, L], F32, kind="ExternalInput")
    dec_d = nc.dram_tensor("dec", [P, 8], F32, kind="ExternalInput")
    emat_d = nc.dram_tensor("emat", [M, 8 * P], F32R, kind="ExternalInput")
    ones1_d = nc.dram_tensor("ones1", [1, P], F32R, kind="ExternalInput")
    pgw_d = nc.dram_tensor("pgw", [D, P], F32R, kind="ExternalInput")
    pgb_d = nc.dram_tensor("pgb", [P, 1], F32, kind="ExternalInput")
    gow_d = nc.dram_tensor("gow", [D, D], F32R, kind="ExternalInput")
    skw_d = nc.dram_tensor("skw", [D, D], F32R, kind="ExternalInput")
    mixw_d = nc.dram_tensor("mixw", [2 * M * C, D], F32R, kind="ExternalInput")
    gob_d = nc.dram_tensor("gob", [1, D], F32R, kind="ExternalInput")
    skb_d = nc.dram_tensor("skb", [1, D], F32R, kind="ExternalInput")
    mixb_d = nc.dram_tensor("mixb", [1, D], F32R, kind="ExternalInput")

    out_d = nc.dram_tensor("out", [L, D], F32, kind="ExternalOutput")
    gx_d = nc.dram_tensor("gx", [M, L], F32R, kind="ExternalOutput")

    with tile.TileContext(nc) as tc:
        with (
            tc.tile_pool(name="const", bufs=1) as cpool,
            tc.tile_pool(name="s", bufs=1) as spool,
        ):
            xt_stack = contextlib.ExitStack()
            xpool = xt_stack.enter_context(
                tc.tile_pool(name="xtp", bufs=1, side="right"))
            # --- long-lived tiles ---
            ones1 = cpool.tile([1, P], F32R, tag="ones1")
            gob_t = cpool.tile([1, D], F32R, tag="gob")
            skb_t = cpool.tile([1, D], F32R, tag="skb")
            mixb_t = cpool.tile([1, D], F32R, tag="mixb")
            gx = cpool.tile([M, L], F32R, tag="gxs")
            nc.sync.dma_start(ones1[:], ones1_d[:])
            nc.sync.dma_start(gob_t[:], gob_d[:])
            nc.sync.dma_start(skb_t[:], skb_d[:])
            nc.sync.dma_start(mixb_t[:], mixb_d[:])

            s_re = [spool.tile([P, L], F32R, tag=f"sre{g}", name=f"sre{g}")
                    for g in range(NG)]
            s_im = [spool.tile([P, L], F32R, tag=f"sim{g}", name=f"sim{g}")
                    for g in range(NG)]

            xt = xpool.tile([P, KD * L], F32R, tag="xt")
            for kc in range(KD):
                nc.sync.dma_start(xt[:, kc * L:(kc + 1) * L],
                                  xt_d[kc * P:(kc + 1) * P, :])

            # =============== phase 1-3: scan path ===============
            with (
                tc.tile_pool(name="scanc", bufs=1, side="right") as scpool,
                tc.tile_pool(name="psA", bufs=1, space="PSUM") as psA,
                tc.tile_pool(name="psR", bufs=2, space="PSUM") as psR,
                tc.tile_pool(name="scant", bufs=1, side="right") as tpool,
                tc.tile_pool(name="rstg", bufs=4, side="right") as rpool,
                tc.tile_pool(name="u", bufs=1, side="right") as upool,
                tc.tile_pool(name="p1", bufs=1, side="right") as p1pool,
            ):
                costab = scpool.tile([P, L], F32, tag="costab")
                nsintab = scpool.tile([P, L], F32, tag="nsintab")
                dec = scpool.tile([P, 8], F32, tag="dec")
                emat = scpool.tile([M, 8 * P], F32R, tag="emat")
                pgw = scpool.tile([P, D], F32R, tag="pgw")
                pgb = scpool.tile([P, 1], F32, tag="pgb")
                keep = scpool.tile([1, L], F32R, tag="keep")
                keeprep = scpool.tile([P, L], F32, tag="keeprep")
                nc.sync.dma_start(costab[:], costab_d[:])
                nc.sync.dma_start(nsintab[:], nsintab_d[:])
                nc.sync.dma_start(dec[:], dec_d[:])
                nc.sync.dma_start(emat[:], emat_d[:])
                nc.sync.dma_start(pgb[:], pgb_d[:])
                nc.sync.dma_start(keep[:], keep_d[:])
                for kc in range(KD):
                    nc.sync.dma_start(pgw[:, kc * P:(kc + 1) * P],
                                      pgw_d[kc * P:(kc + 1) * P, :])

                pgps = psA.tile([P, L], F32, tag="pg")
                for th in range(2):
                    sl = slice(th * 512, (th + 1) * 512)
                    for kc in range(KD):
                        nc.tensor.matmul(
                            pgps[:, sl],
                            lhsT=pgw[:, kc * P:(kc + 1) * P],
                            rhs=xt[:, kc * L + th * 512: kc * L + (th + 1) * 512],
                            start=(kc == 0), stop=(kc == KD - 1))
                pg = p1pool.tile([P, L], F32, tag="pg1")
                gi2 = p1pool.tile([M, L], F32, tag="gi2")
                # pre rows 0:64 (+bias), sigmoid(gi) rows 64:128 (+bias)
                nc.scalar.activation(pg[0:M, :], pgps[0:M, :], AF.Identity,
                                     bias=pgb[0:M, 0:1])
                nc.scalar.activation(pg[M:P, :], pgps[M:P, :], AF.Sigmoid,
                                     bias=pgb[M:P, 0:1])
                nc.sync.dma_start(gi2[:], pg[M:P, :])
                nc.vector.tensor_tensor(gx[:], pg[0:M, :], gi2[:], AL.mult)
                nc.sync.dma_start(gx_d[:], gx[:])

                # keep replication to all 128 partitions
                kps = psA.tile([P, L], F32, tag="keep")
                for th in range(2):
                    nc.tensor.matmul(kps[:, th * 512:(th + 1) * 512],
                                     lhsT=ones1[:],
                                     rhs=keep[:, th * 512:(th + 1) * 512],
                                     start=True, stop=True)
                nc.scalar.copy(keeprep[:], kps[:])

                # per group: replicate gx, scan, recover s
                for g in range(NG):
                    rps = psR.tile([P, L], F32, tag="repl")
                    for th in range(2):
                        nc.tensor.matmul(
                            rps[:, th * 512:(th + 1) * 512],
                            lhsT=emat[:, g * P:(g + 1) * P],
                            rhs=gx[:, th * 512:(th + 1) * 512],
                            start=True, stop=True)
                    rsb = rpool.tile([P, L], F32, tag="rsb", name=f"rsb{g}")
                    nc.scalar.copy(rsb[:], rps[:])
                    b_re = tpool.tile([P, L], F32, tag="bre")
                    b_im = tpool.tile([P, L], F32, tag="bim")
                    a_g = tpool.tile([P, L], F32, tag="ag")
                    nc.vector.tensor_tensor(b_re[:], rsb[:], costab[:], AL.mult)
                    nc.vector.tensor_tensor(b_im[:], rsb[:], nsintab[:], AL.mult)
                    nc.vector.tensor_scalar(a_g[:], keeprep[:], dec[:, g:g + 1],
                                            None, op0=AL.mult)
                    u_re = upool.tile([P, L], F32, tag="ure")
                    u_im = upool.tile([P, L], F32, tag="uim")
                    nc.vector.tensor_tensor_scan(u_re[:], a_g[:], b_re[:], 0.0,
                                                 op0=AL.mult, op1=AL.add)
                    nc.vector.tensor_tensor_scan(u_im[:], a_g[:], b_im[:], 0.0,
                                                 op0=AL.mult, op1=AL.add)
                    # s_re = cos*u_re + (-sin)*u_im
                    # s_im = cos*u_im - (-sin)*u_re
                    t1 = tpool.tile([P, L], F32, tag="t1")
                    t2 = tpool.tile([P, L], F32, tag="t2")
                    nc.vector.tensor_tensor(t1[:], costab[:], u_re[:], AL.mult)
                    nc.vector.tensor_tensor(t2[:], nsintab[:], u_im[:], AL.mult)
                    nc.vector.tensor_tensor(s_re[g][:], t1[:], t2[:], AL.add)
                    nc.vector.tensor_tensor(t1[:], costab[:], u_im[:], AL.mult)
                    nc.vector.tensor_tensor(t2[:], nsintab[:], u_re[:], AL.mult)
                    nc.vector.tensor_tensor(s_im[g][:], t1[:], t2[:], AL.subtract)

            # =============== phase 4: gate_out / skip ===============
            with tc.tile_pool(name="gos", bufs=1) as gpool:
              go_sig = [gpool.tile([P, D], F32, tag=f"gosig{tb}",
                                   name=f"gosig{tb}") for tb in range(NTB)]
              sk1 = [gpool.tile([P, D], F32, tag=f"sk1{tb}",
                                name=f"sk1{tb}") for tb in range(NTB)]
              with (
                tc.tile_pool(name="psB4", bufs=1, space="PSUM") as psB4,
                tc.tile_pool(name="w4", bufs=2, side="right") as wpool4,
                tc.tile_pool(name="gneg", bufs=2, side="right") as npool,
              ):
                # four passes (go half0, go half1, sk half0, sk half1);
                # weights loaded once per (pass, kc); 8 PSUM banks = one per tb
                for pi, (is_go, h) in enumerate(
                        [(True, 0), (True, 1), (False, 0), (False, 1)]):
                    hs = slice(h * 512, (h + 1) * 512)
                    wdram = gow_d if is_go else skw_d
                    brow = gob_t if is_go else skb_t
                    zb = [psB4.tile([P, 512], F32, tag=f"bank{i}",
                                    name=f"b4_{pi}_{i}") for i in range(NTB)]
                    for tb in range(NTB):
                        nc.tensor.matmul(zb[tb][:], lhsT=ones1[:],
                                         rhs=brow[:, hs],
                                         start=True, stop=False)
                    for kc in range(KD):
                        w = wpool4.tile([P, 512], F32R, tag="w4", bufs=3,
                                        name=f"w4_{pi}_{kc}")
                        nc.sync.dma_start(w[:], wdram[kc * P:(kc + 1) * P, hs])
                        for tb in range(NTB):
                            nc.tensor.matmul(
                                zb[tb][:],
                                lhsT=xt[:, kc * L + tb * P: kc * L + (tb + 1) * P],
                                rhs=w[:],
                                start=False, stop=(kc == KD - 1))
                    for tb in range(NTB):
                        if is_go:
                            nc.scalar.activation(go_sig[tb][:, hs], zb[tb][:],
                                                 AF.Sigmoid)
                        else:
                            gneg = npool.tile([P, 512], F32, tag="gneg",
                                              name=f"gneg_{pi}_{tb}")
                            nc.vector.tensor_scalar(gneg[:], go_sig[tb][:, hs],
                                                    -1.0, 1.0, op0=AL.mult,
                                                    op1=AL.add)
                            nc.vector.scalar_tensor_tensor(
                                sk1[tb][:, hs], zb[tb][:], 0.0, gneg[:],
                                op0=AL.bypass, op1=AL.mult)

              # xt no longer needed; free its SBUF before the mix phase
              xt_stack.close()
              # =============== phase 5: mix matmul + zg ===============
              if True:
                with (
                    tc.tile_pool(name="psB5", bufs=1, space="PSUM") as psB5,
                    tc.tile_pool(name="w5", bufs=4) as wpool5,
                    tc.tile_pool(name="zgp", bufs=1) as zpool,
                    tc.tile_pool(name="ep", bufs=2) as epool,
                ):
                    zg = [zpool.tile([P, D], F32, tag=f"zg{tb}", name=f"zg{tb}")
                          for tb in range(NTB)]
                    s1 = [zpool.tile([P, 2], F32, tag=f"s1_{tb}", name=f"s1_{tb}")
                          for tb in range(NTB)]
                    for h in range(2):
                        hs = slice(h * 512, (h + 1) * 512)
                        zb = [psB5.tile([P, 512], F32, tag=f"bank{i}",
                                        name=f"bank5_{h}_{i}") for i in range(NTB)]
                        for tb in range(NTB):
                            nc.tensor.matmul(
                                zb[tb][:], lhsT=ones1[:],
                                rhs=mixb_t[:, h * 512:(h + 1) * 512],
                                start=True, stop=False)
                        for cc in range(KCH):
                            mw = wpool5.tile([P, 512], F32R, tag="w5")
                            nc.sync.dma_start(mw[:], mixw_d[cc * P:(cc + 1) * P, hs])
                            stile = s_re[cc] if cc < NG else s_im[cc - NG]
                            for tb in range(NTB):
                                nc.tensor.matmul(
                                    zb[tb][:],
                                    lhsT=stile[:, tb * P:(tb + 1) * P],
                                    rhs=mw[:],
                                    start=False, stop=(cc == KCH - 1))
                        for tb in range(NTB):
                            nc.vector.scalar_tensor_tensor(
                                zg[tb][:, hs], zb[tb][:], 0.0, go_sig[tb][:, hs],
                                op0=AL.bypass, op1=AL.mult,
                                accum_out=s1[tb][:, h:h + 1])

                    # =============== phase 6: LayerNorm epilogue ===============
                    for tb in range(NTB):
                        st = epool.tile([P, 8], F32, tag="stats")
                        sq = epool.tile([P, D], F32, tag="sq")
                        # st: 0 sum->inv, 1 sumsq, 2 mu, 3 mu^2, 4 sumsq/D,
                        #     5 var, 6 var+eps, 7 sd
                        nc.vector.tensor_tensor(st[:, 0:1], s1[tb][:, 0:1],
                                                s1[tb][:, 1:2], AL.add)
                        nc.scalar.activation(sq[:], zg[tb][:], AF.Square,
                                             accum_out=st[:, 1:2])
                        nc.vector.tensor_scalar(st[:, 2:3], st[:, 0:1], 1.0 / D,
                                                None, op0=AL.mult)
                        nc.vector.tensor_tensor(st[:, 3:4], st[:, 2:3], st[:, 2:3],
                                                AL.mult)
                        nc.vector.tensor_scalar(st[:, 4:5], st[:, 1:2], 1.0 / D,
                                                None, op0=AL.mult)
                        nc.vector.tensor_tensor(st[:, 5:6], st[:, 4:5], st[:, 3:4],
                                                AL.subtract)
                        nc.vector.tensor_scalar(st[:, 6:7], st[:, 5:6], EPS,
                                                None, op0=AL.add)
                        nc.scalar.activation(st[:, 7:8], st[:, 6:7], AF.Sqrt)
                        nc.vector.reciprocal(st[:, 0:1], st[:, 7:8])
                        fin = epool.tile([P, D], F32, tag="fin")
                        nc.vector.tensor_scalar(fin[:], zg[tb][:], st[:, 2:3],
                                                st[:, 0:1], op0=AL.subtract,
                                                op1=AL.mult)
                        nc.vector.tensor_tensor(fin[:], fin[:], sk1[tb][:], AL.add)
                        nc.sync.dma_start(out_d[tb * P:(tb + 1) * P, :], fin[:])

    return nc


# ----------------------------------------------------------------------------
# host-side input prep
# ----------------------------------------------------------------------------

def _prep_host(x, start, pre_w, pre_b, gi_w, gi_b, go_w, go_b,
               skip_w, skip_b, mix_w, mix_b, ffa_a, ffa_b):
    x = np.asarray(x, np.float32)
    start = np.asarray(start)
    a64 = np.abs(np.asarray(ffa_a, np.float64))
    b64 = np.asarray(ffa_b, np.float64)

    xc = x.reshape(NCORES, L, D)
    xT = np.ascontiguousarray(xc.transpose(0, 2, 1))          # [8, D, L]
    keep = np.ascontiguousarray(
        1.0 - start.reshape(NCORES, 1, L).astype(np.float32))

    t64 = np.arange(L, dtype=np.float64)
    c_of_p = np.arange(P) % C
    ang = b64[c_of_p][:, None] * t64[None, :]                 # [128, L]
    costab = np.cos(ang).astype(np.float32)
    nsintab = (-np.sin(ang)).astype(np.float32)

    decay = np.exp(-a64)                                      # [64]
    p_idx = np.arange(P)
    dec = np.zeros((P, 8), np.float32)
    for g in range(8):
        dec[:, g] = decay[g * 8 + p_idx // C]
    emat = np.zeros((M, 8 * P), np.float32)
    for g in range(8):
        emat[g * 8 + p_idx // C, g * P + p_idx] = 1.0

    pgw = np.ascontiguousarray(
        np.concatenate([np.asarray(pre_w, np.float32).T,
                        np.asarray(gi_w, np.float32).T], axis=1))  # [D, 128]
    pgb = np.concatenate([np.asarray(pre_b, np.float32),
                          np.asarray(gi_b, np.float32)])[:, None]  # [128,1]
    gow = np.ascontiguousarray(np.asarray(go_w, np.float32).T)
    skw = np.ascontiguousarray(np.asarray(skip_w, np.float32).T)
    ch = np.arange(M * C)
    cols_re = (ch // C) * (2 * C) + ch % C
    cols_im = cols_re + C
    mwT = np.asarray(mix_w, np.float32).T                     # [2048, 1024]
    mixw = np.ascontiguousarray(
        np.concatenate([mwT[cols_re], mwT[cols_im]], axis=0))
    ones1 = np.ones((1, P), np.float32)

    common = dict(
        costab=costab, nsintab=nsintab, dec=dec, emat=emat, ones1=ones1,
        pgw=pgw, pgb=pgb, gow=gow, skw=skw, mixw=mixw,
        gob=np.asarray(go_b, np.float32)[None, :],
        skb=np.asarray(skip_b, np.float32)[None, :],
        mixb=np.asarray(mix_b, np.float32)[None, :],
    )
    in_maps = []
    for i in range(NCORES):
        m = dict(common)
        m["xt"] = xT[i]
        m["keep"] = keep[i]
        in_maps.append(m)
    return in_maps


# ----------------------------------------------------------------------------
# host-side carry fix-up
# ----------------------------------------------------------------------------

def _fixup(out, gx_chunks, x, start, state_re, state_im,
           go_w, go_b, skip_w, skip_b, mix_w, mix_b, ffa_a, ffa_b):
    """Apply the cross-chunk carried-state correction to the prefix rows of
    each chunk (rows before the chunk's first episode reset) and compute the
    final carried state.  All in float64 on host; touches O(cores) rows for
    Bernoulli start flags."""
    a64 = np.abs(np.asarray(ffa_a, np.float64))
    b64 = np.asarray(ffa_b, np.float64)
    gamma = np.exp(-a64)[:, None] * np.exp(1j * b64)[None, :]   # [64, 16]
    log_decay = -a64[:, None]

    startc = np.asarray(start).reshape(NCORES, L)
    x64 = np.asarray(x, np.float64)
    goW = np.asarray(go_w, np.float64); goB = np.asarray(go_b, np.float64)
    skW = np.asarray(skip_w, np.float64); skB = np.asarray(skip_b, np.float64)
    mxW = np.asarray(mix_w, np.float64); mxB = np.asarray(mix_b, np.float64)

    def gamma_pow(k):
        return np.exp(log_decay * k) * np.exp(1j * b64[None, :] * k)

    # zero-carry chunk-final states S_i from device gx
    S = []
    for i in range(NCORES):
        gxi = np.asarray(gx_chunks[i], np.float64)              # [64, L]
        s_i = startc[i]
        j0 = int(np.flatnonzero(s_i)[-1]) if s_i.any() else 0
        js = np.arange(j0, L)
        expo = (L - 1 - js)
        E1 = np.exp(log_decay * expo[None, :])                  # [64, nj]
        E2 = np.exp(1j * b64[:, None] * expo[None, :])          # [16, nj]
        S.append(np.einsum('mj,mj,cj->mc', E1, gxi[:, js], E2))
    # carry chain
    Cs = [np.asarray(state_re, np.float64)[0] +
          1j * np.asarray(state_im, np.float64)[0]]             # [64,16]
    for i in range(NCORES):
        if startc[i].any():
            Cs.append(S[i])
        else:
            Cs.append(gamma_pow(L) * Cs[i] + S[i])

    # recompute prefix rows with the carried state
    for i in range(NCORES):
        nz = np.flatnonzero(startc[i])
        Pfx = int(nz[0]) if nz.size else L
        if Pfx == 0 or not np.any(np.abs(Cs[i]) > 0):
            continue
        gxi = np.asarray(gx_chunks[i], np.float64)
        s_t = Cs[i].copy()
        for t in range(Pfx):
            s_t = gamma * s_t + gxi[:, t][:, None]
            zrow = np.concatenate([s_t.real, s_t.imag], axis=1).reshape(-1)
            z = mxW @ zrow + mxB
            xr = x64[i * L + t]
            gate = 1.0 / (1.0 + np.exp(-(goW @ xr + goB)))
            skip = skW @ xr + skB
            zg = z * gate
            mu = zg.mean()
            var = ((zg - mu) ** 2).mean()
            ln = (zg - mu) / np.sqrt(var + EPS)
            out[i * L + t] = (ln + skip * (1.0 - gate)).astype(np.float32)

    final = Cs[NCORES].astype(np.complex64)[None, :, :]         # [1, 64, 16]
    return out, final


# ----------------------------------------------------------------------------
# runner (jitted shard_map over the bass_exec custom call), cached
# ----------------------------------------------------------------------------
_cache = {}


def _get_runner():
    if "fn" in _cache:
        return _cache["fn"], _cache["meta"]
    _install_patches()
    import jax
    from jax.sharding import Mesh, PartitionSpec
    from jax.experimental.shard_map import shard_map
    from concourse.bass2jax import (_bass_exec_p, partition_id_tensor,
                                    install_neuronx_cc_hook)
    install_neuronx_cc_hook()

    nc = build_program()

    in_names, out_names, out_avals = [], [], []
    partition_name = nc.partition_id_tensor.name if nc.partition_id_tensor else None
    for alloc in nc.m.functions[0].allocations:
        if not isinstance(alloc, mybir.MemoryLocationSet):
            continue
        name = alloc.memorylocations[0].name
        if alloc.kind == "ExternalInput":
            if name != partition_name:
                in_names.append(name)
        elif alloc.kind == "ExternalOutput":
            out_names.append(name)
            out_avals.append(jax.core.ShapedArray(
                tuple(alloc.tensor_shape), mybir.dt.np(alloc.dtype)))
    n_params = len(in_names)
    all_in = tuple(in_names + out_names +
                   ([partition_name] if partition_name else []))

    def _body(*args):
        operands = list(args)
        if partition_name is not None:
            operands.append(partition_id_tensor())
        outs = _bass_exec_p.bind(
            *operands,
            out_avals=tuple(out_avals),
            in_names=all_in,
            out_names=tuple(out_names),
            lowering_input_output_aliases=(),
            sim_require_finite=True,
            sim_require_nnan=True,
            nc=nc,
        )
        return tuple(outs)

    devices = jax.devices()[:NCORES]
    mesh = Mesh(np.asarray(devices), ("core",))
    nin = n_params + len(out_names)
    fn = jax.jit(shard_map(_body, mesh=mesh,
                           in_specs=(PartitionSpec("core"),) * nin,
                           out_specs=(PartitionSpec("core"),) * len(out_names),
                           check_rep=False))
    meta = (in_names, out_names, out_avals)
    _cache["fn"] = fn
    _cache["meta"] = meta
    return fn, meta


def run_device(in_maps):
    import jax
    fn, (in_names, out_names, out_avals) = _get_runner()
    concat_in = [
        np.concatenate([np.asarray(in_maps[c][name]) for c in range(NCORES)],
                       axis=0)
        for name in in_names
    ]
    concat_zeros = [
        np.zeros((NCORES * a.shape[0], *a.shape[1:]), a.dtype)
        for a in out_avals
    ]
    outs = fn(*concat_in, *concat_zeros)
    jax.block_until_ready(outs)
    res = {}
    for i, name in enumerate(out_names):
        res[name] = np.asarray(outs[i]).reshape(NCORES, *out_avals[i].shape)
    return res


# ----------------------------------------------------------------------------
# public entry point
# ----------------------------------------------------------------------------

def kernel(x, state_re, state_im, start, next_done,
           pre_w, pre_b, gi_w, gi_b, go_w, go_b,
           skip_w, skip_b, mix_w, mix_b, ffa_a, ffa_b):
    in_maps = _prep_host(x, start, pre_w, pre_b, gi_w, gi_b, go_w, go_b,
                         skip_w, skip_b, mix_w, mix_b, ffa_a, ffa_b)
    res = run_device(in_maps)
    out = res["out"].reshape(T, D).astype(np.float32).copy()
    gx_chunks = [res["gx"][i] for i in range(NCORES)]
    out, final = _fixup(out, gx_chunks, x, start, state_re, state_im,
                        go_w, go_b, skip_w, skip_b, mix_w, mix_b,
                        ffa_a, ffa_b)
    return out, final
